# revision 79
# baseline (speedup 1.0000x reference)
"""Trainium2 Bass kernel for CustomRobertaAttention (B=4, S=2048, H=1024, NH=16).

Sharding: 8 cores = (batch b, query-half qh), zero cross-core comms; host
permutes tokens per core so local queries are rows 0..1023 and gathers.

Numerics: fp8e4 (e4m3) DoubleRow matmuls everywhere (2x128 contraction packed
per instruction). Host pre-casts x and 16*W to fp8. Scale/bias folding:
  - bk dropped (softmax-invariant along keys), bq folded into Q drain
  - bv/bo folded into the residual stream (x'' = x + bo + bv@Wo, host side)
  - attention mask folded into V via em=exp(mask) (V drain scale + ones col)
  - exp arg scale 1/2048 absorbs the 16*16 weight prescale and 1/sqrt(64)
Softmax exp runs on ACT (true exp->fp8) and DVE (Schraudolph bit-trick
ts->int8 emitting the fp8 bit pattern); only these two engines can read PSUM,
so they are the bottleneck and all SBUF-side work (residual add, LN apply)
is pushed to Pool. When the inputs have ln_gamma==1 and ln_beta==0 (the
shipped setup_inputs), the gamma/beta ops are folded away entirely.
LayerNorm: bn_stats + batched Newton rsqrt.
"""
import numpy as np

import concourse.bass as bass
import concourse.bacc as bacc
import concourse.tile as tile
import concourse.mybir as mybir

F32 = mybir.dt.float32
BF16 = mybir.dt.bfloat16
FP8 = mybir.dt.float8e4
I8 = mybir.dt.int8
EXP = mybir.ActivationFunctionType.Exp
IDN = mybir.ActivationFunctionType.Identity
OP = mybir.AluOpType
DR = mybir.MatmulPerfMode.DoubleRow

P = 128
HD = 64
LN_EPS = 1e-12
LOG2E = 1.4426950408889634

# ---- engine assignment knobs -------------------------------------------------
# exp engine patterns: 8 kt-pair slots per unit; "A"=ACT true exp,
# "D"=DVE Schraudolph bit-trick.
EXP_PAT_QB0 = (("A", "D", "A", "D", "A", "A", "D", "A"),
               ("D", "A", "A", "D", "A", "A", "D", "A"))
EXP_PAT_QB1 = (("A", "D", "A", "A", "D", "A", "D", "A"),
               ("D", "A", "A", "D", "A", "A", "D", "A"))
EXP_PAT_LAST = ("A", "A", "D", "A", "A", "A", "D", "A")
V_ROT = ("A", "D")       # V projection pair-drain engines
K_ROT = ("A", "D")       # K projection pair-drain engines
Q_ROT = ("A", "D")       # Q projection pair-drain engines
EPI_DRAIN = ("A", "A")   # out-proj drain engine per 512-col half
APPLY_ENG = "D"          # LN apply engine (fold path)
RESID_ENG = "D"          # residual add engine
TAIL_RESID_ENG = "D"
TAIL_APPLY_ENG = "A"


class Cfg:
    def __init__(self, S, SQ, H, NH):
        self.S, self.SQ, self.H, self.NH = S, SQ, H, NH
        self.HT = H // P        # 8 h-tiles
        self.KS = self.HT // 2  # 4 DoubleRow contraction steps
        self.DT = H // P        # 8 d-tiles
        self.NKT = S // P       # 16 key tiles
        self.QB = 512
        self.NQB = SQ // self.QB
        self.NQT = SQ // P      # 8 query P-tiles


def _eng(nc, tag):
    return {"A": nc.scalar, "D": nc.vector, "P": nc.gpsimd}[tag]


def _drain(nc, tag, out, in0, mul=None, add=None):
    """PSUM->SBUF cast with optional per-partition [P,1] AP scale/bias.
    tag 'A' -> ACT Identity(in*mul + add); 'D' -> tensor_scalar."""
    if tag == "A":
        nc.scalar.activation(out, in0, IDN,
                             bias=add if add is not None else 0.0,
                             scale=mul if mul is not None else 1.0)
    else:
        eng = _eng(nc, tag)
        if mul is not None and add is not None:
            eng.tensor_scalar(out=out, in0=in0, scalar1=mul, scalar2=add,
                              op0=OP.mult, op1=OP.add)
        elif mul is not None:
            eng.tensor_scalar(out=out, in0=in0, scalar1=mul, scalar2=None,
                              op0=OP.mult)
        elif add is not None:
            eng.tensor_scalar(out=out, in0=in0, scalar1=add, scalar2=None,
                              op0=OP.add)
        else:
            eng.tensor_copy(out, in0)


def _consts(nc, cf, io, consts, fold_gb):
    cs = {}
    cs["em_t"] = consts.tile([P, cf.NKT], F32, name="em_t")
    nc.sync.dma_start(cs["em_t"][:], io["em_t"][:, :])
    cs["em8"] = consts.tile([P, cf.NKT, cf.NH], FP8, name="em8")
    nc.sync.dma_start(cs["em8"][:], io["em8"][:, :, :])
    cs["bq16"] = consts.tile([P, cf.DT], F32, name="bq16")
    nc.sync.dma_start(cs["bq16"][:], io["bq16"][:, :])
    if not fold_gb:
        for nm in ("gamma", "beta"):
            cs[nm] = consts.tile([P, cf.H], F32, name=nm)
            nc.sync.dma_start(cs[nm][:],
                              io[nm].ap().unsqueeze(0).partition_broadcast(P))
    return cs


def _v_jobs(nc, cf, io, cs, XT, W, V8, pools):
    """Build per-kt V projection closures: one [P,2,512] psum pair (shared
    psS ring) -> one 1024-col drain. Issued interleaved into units 0-1 so PE
    never stalls on a monolithic V phase. Also writes the em ones columns."""
    for kt in range(cf.NKT):
        nc.gpsimd.tensor_copy(V8[:, kt, :, HD], cs["em8"][:, kt, :])

    def mk(kt):
        def job():
            ps = pools["psS"].tile([P, 2, 512], F32, tag="psS")
            for dc in range(2):
                for ks in range(cf.KS):
                    nc.tensor.matmul(
                        ps[:, dc, :],
                        XT[:, 2 * ks:2 * ks + 2, kt * P:(kt + 1) * P],
                        W["wv"][:, ks, :, dc * 512:(dc + 1) * 512],
                        start=(ks == 0), stop=(ks == cf.KS - 1), perf_mode=DR)
            tag = V_ROT[kt % len(V_ROT)]
            _drain(nc, tag,
                   V8[:, kt, :, 0:HD],
                   ps[:].rearrange("p a b -> p (a b)")
                        .rearrange("p (h d) -> p h d", d=HD),
                   mul=cs["em_t"][:, kt:kt + 1])
        return job

    return [mk(kt) for kt in range(cf.NKT)]


def _kq_jobs(nc, cf, cs, W, XT, KTsc, QTsc, pools, dt):
    """K+Q projection jobs for one d-tile; paired 1024-col drains; merged
    scatter. Returns 4 closures so PE work interleaves with unit slots."""
    state = {}

    def k_pair(c2):
        def job():
            if "ktsb" not in state:
                state["ktsb"] = pools["ktsb"].tile([P, cf.S], FP8, tag="ktsb", name=f"ktsb{dt}")
            ktsb = state["ktsb"]
            ps = pools["psS"].tile([P, 2, 512], F32, tag="psS")
            for j2 in range(2):
                c = 2 * c2 + j2
                for ks in range(cf.KS):
                    nc.tensor.matmul(
                        ps[:, j2, :], W["wk"][:, dt, ks, :, :],
                        XT[:, 2 * ks:2 * ks + 2, c * 512:(c + 1) * 512],
                        start=(ks == 0), stop=(ks == cf.KS - 1), perf_mode=DR)
            _drain(nc, K_ROT[(dt + c2) % 2],
                   ktsb[:, c2 * 1024:(c2 + 1) * 1024],
                   ps[:].rearrange("p a b -> p (a b)"))
        return job

    def q_pair():
        def job():
            state["qtsb"] = pools["qtsb"].tile([P, cf.SQ], FP8, tag="qtsb", name=f"qtsb{dt}")
            qtsb = state["qtsb"]
            ps = pools["psS"].tile([P, 2, 512], F32, tag="psS")
            for j2 in range(2):
                for ks in range(cf.KS):
                    nc.tensor.matmul(
                        ps[:, j2, :], W["wq"][:, dt, ks, :, :],
                        XT[:, 2 * ks:2 * ks + 2, j2 * 512:(j2 + 1) * 512],
                        start=(ks == 0), stop=(ks == cf.KS - 1), perf_mode=DR)
            _drain(nc, Q_ROT[dt % 2],
                   qtsb[:, 0:1024],
                   ps[:].rearrange("p a b -> p (a b)"),
                   add=cs["bq16"][:, dt:dt + 1])
        return job

    def scatter():
        def job():
            for hp in range(2):
                h = 2 * dt + hp
                w, g = h % 4, h // 4
                nc.gpsimd.dma_start(KTsc[32 * w:32 * (w + 1), g, :, :],
                                    state["ktsb"][64 * hp:64 * hp + 64, :])
                nc.sync.dma_start(QTsc[32 * w:32 * (w + 1), g, :, :],
                                  state["qtsb"][64 * hp:64 * hp + 64, :])
        return job

    return [k_pair(0), k_pair(1), q_pair(), scatter()]


def _kq_dt(nc, cf, cs, W, XT, KTsc, QTsc, pools, dt):
    for job in _kq_jobs(nc, cf, cs, W, XT, KTsc, QTsc, pools, dt):
        job()


def _unit_step(nc, cf, pools, cs, KTsc, QTsc, V8, ctxT, h, qb, prev,
               vjobs=None, last=False, slot_jobs=None):
    """One attention unit, fine-grained: per pair-slot emit 2 score matmuls +
    exp + one ctx step of the PREVIOUS unit. Returns (h, qb, pairs, psc)."""
    w, g = h % 4, h // 4
    q0 = qb * cf.QB
    a_dve = 8.0 * LOG2E / 2048.0
    pats = EXP_PAT_QB0 if qb == 0 else EXP_PAT_QB1
    pat = pats[h % len(pats)]
    if last:
        pat = EXP_PAT_LAST
    slot_jobs = slot_jobs or {}
    pairs = []
    psc = pools["psC"].tile([HD + 1, 512], F32, tag="psC",
                            name=f"psc_{qb}_{h}")
    for i in range(8):
        tag = pat[i]
        pss = pools["psS"].tile([P, 2, 512], F32, tag="psS")
        for j in range(2):
            kt = 2 * i + j
            nc.tensor.matmul(
                pss[:, j, :],
                KTsc[32 * w:32 * (w + 1), g, :, kt * P:(kt + 1) * P],
                QTsc[32 * w:32 * (w + 1), g, :, q0:q0 + cf.QB],
                start=True, stop=True, perf_mode=DR,
                tile_position=(32 * w, 0))
        et = pools["exp"].tile([P, 2, 512], FP8, tag="exp")
        if tag == "A":
            nc.scalar.activation(
                et[:].rearrange("p a b -> p (a b)"),
                pss[:].rearrange("p a b -> p (a b)"),
                EXP, bias=0.0, scale=1.0 / 2048.0)
        else:
            nc.vector.tensor_scalar(
                out=et[:].rearrange("p a b -> p (a b)").bitcast(I8),
                in0=pss[:].rearrange("p a b -> p (a b)"),
                scalar1=a_dve, scalar2=56.0, op0=OP.mult, op1=OP.add)
        pairs.append(et[:])
        if vjobs:
            vjobs.pop(0)()
        for fn in slot_jobs.get(i, ()):
            fn()
        if prev is not None:
            ph, pqb, ppairs, ppsc = prev
            nc.tensor.matmul(
                ppsc[:], V8[:, 2 * i:2 * i + 2, ph, :], ppairs[i],
                start=(i == 0), stop=(i == 7), perf_mode=DR)
    return (h, qb, pairs, psc)


def _ctx_tail(nc, cf, pools, cs, V8, ctxT, prev):
    """Drain the last pending unit's ctx."""
    ph, pqb, ppairs, ppsc = prev
    for i in range(8):
        nc.tensor.matmul(
            ppsc[:], V8[:, 2 * i:2 * i + 2, ph, :], ppairs[i],
            start=(i == 0), stop=(i == 7), perf_mode=DR)


def _norm_start(nc, cf, pools, cs, unit):
    """Denominator -> SBUF -> recip -> DRAM -> bcast DMA.
    Returns the in-flight bcast tile; the TT-norm runs one unit later so the
    DMA chain never blocks an engine queue."""
    h, qb, _, psc = unit
    rec = pools["norm"].tile([1, 512], F32, tag="rec")
    nc.vector.reciprocal(rec[:], psc[HD:HD + 1, :])
    dst = pools["dram"].tile([1, 512], F32, tag="dst")
    nc.sync.dma_start(dst[:], rec[:])
    bc = pools["bcast"].tile([HD, 512], F32, tag="bc")
    nc.sync.dma_start(bc[:], dst[:].rearrange("a k -> (a k)")
                      .unsqueeze(0).partition_broadcast(HD))
    return bc


def _norm_finish(nc, cf, pools, ctxT, unit, bc):
    h, qb, _, psc = unit
    q0 = qb * cf.QB
    dt, hb = h // 2, 64 * (h % 2)
    if hb == 0:
        nc.vector.tensor_tensor(
            out=ctxT[0:HD, dt, q0:q0 + cf.QB], in0=psc[0:HD, :], in1=bc[:],
            op=OP.mult)
    else:
        ctmp = pools["norm"].tile([HD, 512], FP8, tag="ctmp")
        nc.vector.tensor_tensor(out=ctmp[:], in0=psc[0:HD, :], in1=bc[:],
                                op=OP.mult)
        nc.sync.dma_start(ctxT[HD:P, dt, q0:q0 + cf.QB], ctmp[:])


def _epi_oproj_pre(nc, cf, W, ctxT, pools, qt):
    """First 3 accumulation steps of the out-projection (dt 0-5 only) —
    issued early in the tail so PE stays warm while the last norms land."""
    ps = pools["psE"].tile([P, 2, 512], F32, tag="psS", name=f"pse{qt}")
    for c in range(2):
        for s in range(3):
            nc.tensor.matmul(
                ps[:, c, :], ctxT[:, 2 * s:2 * s + 2, qt * P:(qt + 1) * P],
                W["wo"][:, s, :, c * 512:(c + 1) * 512],
                start=(s == 0), stop=False, perf_mode=DR)
    return ps


def _epi_qt(nc, cf, io, cs, W, ctxT, pools, qt, grp, qi, tail=False,
            ps_pre=None):
    """Out-projection + residual for one query P-tile (stats issued
    separately via _epi_bn so the in-order DVE queue never stalls on the
    Pool residual add)."""
    psE = pools["psE"]
    yt, mv = grp
    if ps_pre is not None:
        ps = ps_pre
        for c in range(2):
            nc.tensor.matmul(
                ps[:, c, :], ctxT[:, 6:8, qt * P:(qt + 1) * P],
                W["wo"][:, 3, :, c * 512:(c + 1) * 512],
                start=False, stop=True, perf_mode=DR)
    else:
        ps = psE.tile([P, 2, 512], F32, tag="psS")
        for c in range(2):
            for s in range(4):
                nc.tensor.matmul(
                    ps[:, c, :], ctxT[:, 2 * s:2 * s + 2, qt * P:(qt + 1) * P],
                    W["wo"][:, s, :, c * 512:(c + 1) * 512],
                    start=(s == 0), stop=(s == 3), perf_mode=DR)
    for c in range(2):
        _drain(nc, EPI_DRAIN[c], yt[:, qi, c * 512:(c + 1) * 512],
               ps[:, c, :], mul=1.0 / 256.0)
    xr = pools["xres"].tile([P, cf.H], BF16, tag="xr")
    nc.sync.dma_start(xr[:], io["xres"][qt * P:(qt + 1) * P, :])
    if tail:
        rtag = ("D", "D", "D", "D")[qi % 4]
    else:
        rtag = RESID_ENG
    _eng(nc, rtag).tensor_tensor(out=yt[:, qi, :], in0=yt[:, qi, :],
                                 in1=xr[:], op=OP.add)
    if tail:
        _epi_bn(nc, cf, pools, grp, qi)


def _epi_bn(nc, cf, pools, grp, qi):
    yt, mv = grp
    stats = pools["mv"].tile([P, 2, nc.vector.BN_STATS_DIM], F32, tag="st")
    yv = yt[:, qi, :].rearrange("p (s f) -> p s f", s=2)
    for s in range(2):
        nc.vector.bn_stats(out=stats[:, s, :], in_=yv[:, s, :])
    nc.vector.bn_aggr(out=mv[:, qi, :], in_=stats[:])


def _epi_apply(nc, cf, io, cs, pools, grp, qts, fold_gb, tail=False):
    """Newton rsqrt (batched over len(qts) query tiles) + LN apply
    (+ gamma/beta when not folded) + out DMA."""
    yt, mv = grp
    n = len(qts)
    npool = pools["newt"]
    xv = npool.tile([P, n], F32, tag="xv", name=f"xv{qts[0]}")
    nc.vector.tensor_scalar(out=xv[:], in0=mv[:, 0:n, 1], scalar1=-0.5,
                            scalar2=None, op0=OP.mult)
    rstd = npool.tile([P, n], F32, tag="rstd", name=f"rstd{qts[0]}")
    nc.vector.memset(rstd[:], 1.0)
    tmp = npool.tile([P, n], F32, tag="tmp", name=f"tmp{qts[0]}")
    for _ in range(3):
        nc.vector.tensor_mul(tmp[:], rstd[:], rstd[:])
        nc.vector.tensor_mul(tmp[:], tmp[:], xv[:])
        nc.vector.tensor_scalar(out=tmp[:], in0=tmp[:], scalar1=1.5,
                                scalar2=None, op0=OP.add)
        nc.vector.tensor_mul(rstd[:], rstd[:], tmp[:])
    bias_t = npool.tile([P, n], F32, tag="bias_t", name=f"bias_t{qts[0]}")
    nc.vector.tensor_scalar(out=bias_t[:], in0=mv[:, 0:n, 0], scalar1=-1.0,
                            scalar2=None, op0=OP.mult)
    nc.vector.tensor_mul(bias_t[:], bias_t[:], rstd[:])
    for qi, qt in enumerate(qts):
        if tail:
            atag = ("D", "D")[qi % 2]
        else:
            atag = APPLY_ENG
        ob = pools["outp"].tile([P, cf.H], BF16, tag="ob", name=f"ob{qt}")
        if fold_gb:
            _drain(nc, atag, ob[:], yt[:, qi, :],
                   mul=rstd[:, qi:qi + 1], add=bias_t[:, qi:qi + 1])
        else:
            _drain(nc, atag, yt[:, qi, :], yt[:, qi, :],
                   mul=rstd[:, qi:qi + 1], add=bias_t[:, qi:qi + 1])
            nc.gpsimd.tensor_mul(yt[:, qi, :], yt[:, qi, :], cs["gamma"][:])
            nc.vector.tensor_tensor(out=ob[:], in0=yt[:, qi, :],
                                    in1=cs["beta"][:], op=OP.add)
        q = (nc.gpsimd if (tail and qi % 2 == 1) else nc.sync)
        q.dma_start(io["out"][qt * P:(qt + 1) * P, :], ob[:])


def _attn_epi(nc, cf, io, cs, KTsc, QTsc, V8, ctxT, pools, W, XT, fold_gb,
              vjobs):
    """Attention units (fine-grained pipeline) with qb0 epilogues interleaved
    into qb1, then the tail epilogues in pipelined pairs."""
    ypool, mvpool = pools["y"], pools["mv"]

    def new_grp(i):
        return (ypool.tile([P, 2, cf.H], BF16, name=f"y{i}", tag="y"),
                mvpool.tile([P, 2, 2], F32, name=f"mvg{i}", tag="mv"))

    grps = {}
    prev = None           # unit whose ctx interleaves into the current one
    norm_q = []           # units with ctx done, norm chain in flight
    pending = {}          # global unit idx -> {slot: [fns]} deferred work
    NU = cf.NQB * cf.NH

    def schedule(g, slot, fn):
        if g >= NU:
            fn()          # past the last unit: run at once (pre-tail)
        else:
            pending.setdefault(g, {}).setdefault(slot, []).append(fn)

    # qb1 runs odd heads first so the final units' norms skip the ctmp DMA
    # hop (even heads write ctxT directly).
    h_orders = (tuple(range(cf.NH)),
                tuple(range(1, cf.NH, 2)) + tuple(range(0, cf.NH, 2)))
    for qb in range(cf.NQB):
        for ui, h in enumerate(h_orders[qb]):
            g = qb * cf.NH + ui
            if qb == 0 and h % 2 == 0 and h // 2 + 1 < cf.DT:
                _kq_dt(nc, cf, cs, W, XT, KTsc, QTsc, pools, h // 2 + 1)
            cur = _unit_step(nc, cf, pools, cs, KTsc, QTsc, V8, ctxT,
                             h, qb, prev, vjobs=vjobs,
                             last=(qb == 1 and ui >= 14),
                             slot_jobs=pending.pop(g, None))
            if prev is not None:
                norm_q.append((prev, _norm_start(nc, cf, pools, cs, prev)))
            if len(norm_q) > 2:
                u, bc = norm_q.pop(0)
                schedule(g + 1, 3,
                         (lambda uu, bb: lambda: _norm_finish(
                             nc, cf, pools, ctxT, uu, bb))(u, bc))
            prev = cur
            if qb == 1 and ui % 4 == 3:
                qt = ui // 4
                pair = qt // 2
                if qt % 2 == 0:
                    grps[pair] = new_grp(pair)
                schedule(g + 1, 5,
                         (lambda q, gr: lambda: _epi_qt(
                             nc, cf, io, cs, W, ctxT, pools, q, gr,
                             q % 2))(qt, grps[pair]))
                schedule(g + 2, 2,
                         (lambda q, gr: lambda: _epi_bn(
                             nc, cf, pools, gr, q % 2))(qt, grps[pair]))
                if qt % 2 == 1:
                    schedule(g + 2, 6,
                             (lambda q, gr: lambda: _epi_apply(
                                 nc, cf, io, cs, pools, gr, (q - 1, q),
                                 fold_gb))(qt, grps[pair]))
    for g in sorted(pending):
        for slot in sorted(pending[g]):
            for fn in pending[g][slot]:
                fn()
    pending.clear()
    _ctx_tail(nc, cf, pools, cs, V8, ctxT, prev)
    pre = {}
    norm_q.append((prev, _norm_start(nc, cf, pools, cs, prev)))
    for u, bc in norm_q:
        _norm_finish(nc, cf, pools, ctxT, u, bc)
    tgrp = (pools["ytail"].tile([P, 4, cf.H], BF16, name="ytail"),
            pools["mv"].tile([P, 4, 2], F32, name="mvtail", tag="mvt"))
    tg0 = (tgrp[0][:, 0:2, :], tgrp[1][:, 0:2, :])
    tg1 = (tgrp[0][:, 2:4, :], tgrp[1][:, 2:4, :])
    for qt in (4, 5):
        _epi_qt(nc, cf, io, cs, W, ctxT, pools, qt, tg0, qt - 4, tail=True,
                ps_pre=pre.get(qt))
    _epi_apply(nc, cf, io, cs, pools, tg0, (4, 5), fold_gb, tail=True)
    for qt in (6, 7):
        _epi_qt(nc, cf, io, cs, W, ctxT, pools, qt, tg1, qt - 6, tail=True,
                ps_pre=pre.get(qt))
    _epi_apply(nc, cf, io, cs, pools, tg1, (6, 7), fold_gb, tail=True)


def build_nc(S=2048, SQ=1024, H=1024, NH=16, QB=512, num_devices=8,
             fold_gb=True):
    cf = Cfg(S, SQ, H, NH)
    nc = bacc.Bacc("TRN2", target_bir_lowering=False, debug=False,
                   num_devices=num_devices)

    io = {}
    io["xt8"] = nc.dram_tensor("xt8", [P, cf.HT, S], FP8, kind="ExternalInput")
    io["wk"] = nc.dram_tensor("wk", [P, cf.DT, cf.KS, 2, P], FP8,
                              kind="ExternalInput")
    io["wq"] = nc.dram_tensor("wq", [P, cf.DT, cf.KS, 2, P], FP8,
                              kind="ExternalInput")
    io["wv"] = nc.dram_tensor("wv", [P, cf.KS, 2, H], FP8,
                              kind="ExternalInput")
    io["wo"] = nc.dram_tensor("wo", [P, cf.KS, 2, H], FP8,
                              kind="ExternalInput")
    io["em_t"] = nc.dram_tensor("em_t", [P, cf.NKT], F32, kind="ExternalInput")
    io["em8"] = nc.dram_tensor("em8", [P, cf.NKT, NH], FP8,
                               kind="ExternalInput")
    io["bq16"] = nc.dram_tensor("bq16", [P, cf.DT], F32, kind="ExternalInput")
    io["xres"] = nc.dram_tensor("xres", [SQ, H], BF16, kind="ExternalInput")
    if not fold_gb:
        io["gamma"] = nc.dram_tensor("gamma", [H], F32, kind="ExternalInput")
        io["beta"] = nc.dram_tensor("beta", [H], F32, kind="ExternalInput")
    io["out"] = nc.dram_tensor("out", [SQ, H], BF16, kind="ExternalOutput")

    with tile.TileContext(nc) as tc, \
         tc.tile_pool(name="consts", bufs=1) as consts, \
         tc.tile_pool(name="xt", bufs=1) as xt_pool, \
         tc.tile_pool(name="wsb", bufs=1) as wsb, \
         tc.tile_pool(name="ktsc", bufs=1) as ktsc_pool, \
         tc.tile_pool(name="qtsc", bufs=1) as qtsc_pool, \
         tc.tile_pool(name="vv", bufs=1) as v_pool, \
         tc.tile_pool(name="ctx", bufs=1) as ctx_pool, \
         tc.tile_pool(name="dram", bufs=4, space="DRAM") as dram_pool:
        XT = xt_pool.tile([P, cf.HT, S], FP8)
        W = {}
        for nm in ("wv", "wk", "wq", "wo"):
            shp = [P, cf.DT, cf.KS, 2, P] if nm in ("wk", "wq") \
                else [P, cf.KS, 2, H]
            W[nm] = wsb.tile(shp, FP8, name=nm)
        # load order tuned for fast start: xt chunk0 + dt0 K/Q weights first
        # so the dt0 K/Q projections can begin ASAP, then V, then the rest.
        nc.sync.dma_start(XT[:, :, 0:512], io["xt8"][:, :, 0:512])
        nc.sync.dma_start(W["wk"][:, 0:1, :, :, :], io["wk"][:, 0:1, :, :, :])
        nc.sync.dma_start(W["wq"][:, 0:1, :, :, :], io["wq"][:, 0:1, :, :, :])
        for c in range(1, 4):
            nc.sync.dma_start(XT[:, :, c * 512:(c + 1) * 512],
                              io["xt8"][:, :, c * 512:(c + 1) * 512])
        nc.sync.dma_start(W["wv"][:], io["wv"][...])
        cs = _consts(nc, cf, io, consts, fold_gb)
        nc.sync.dma_start(W["wk"][:, 1:4, :, :, :], io["wk"][:, 1:4, :, :, :])
        nc.sync.dma_start(W["wq"][:, 1:4, :, :, :], io["wq"][:, 1:4, :, :, :])
        nc.sync.dma_start(W["wk"][:, 4:8, :, :, :], io["wk"][:, 4:8, :, :, :])
        nc.sync.dma_start(W["wq"][:, 4:8, :, :, :], io["wq"][:, 4:8, :, :, :])
        nc.sync.dma_start(W["wo"][:], io["wo"][...])
        KTsc = ktsc_pool.tile([P, 4, 2, S], FP8)
        QTsc = qtsc_pool.tile([P, 4, 2, SQ], FP8)
        V8 = v_pool.tile([P, cf.NKT, NH, HD + 1], FP8)
        ctxT = ctx_pool.tile([P, cf.DT, SQ], FP8)
        pools = {"dram": dram_pool}

        from contextlib import ExitStack
        with ExitStack() as stack:
            psS = stack.enter_context(tc.tile_pool(name="psS", bufs=3, space="PSUM"))
            psC = stack.enter_context(tc.tile_pool(name="psC", bufs=2, space="PSUM"))
            exp_pool = stack.enter_context(tc.tile_pool(name="exp", bufs=24))
            ktsb_pool = stack.enter_context(tc.tile_pool(name="ktsb", bufs=2))
            qtsb_pool = stack.enter_context(tc.tile_pool(name="qtsb", bufs=2))
            norm_pool = stack.enter_context(tc.tile_pool(name="norm", bufs=4))
            bcast_pool = stack.enter_context(tc.tile_pool(name="bcast", bufs=4))
            ypool = stack.enter_context(tc.tile_pool(name="y", bufs=2))
            ytail_pool = stack.enter_context(tc.tile_pool(name="ytail", bufs=1))
            xpool = stack.enter_context(tc.tile_pool(name="xres", bufs=3))
            mvpool = stack.enter_context(tc.tile_pool(name="mv", bufs=3))
            npool = stack.enter_context(tc.tile_pool(name="newt", bufs=2))
            outp = stack.enter_context(tc.tile_pool(name="outp", bufs=4))
            pools.update({"ktsb": ktsb_pool, "qtsb": qtsb_pool})
            pools.update({"psS": psS, "psC": psC, "psE": psS,
                          "exp": exp_pool, "norm": norm_pool,
                          "bcast": bcast_pool, "y": ypool, "xres": xpool,
                          "ytail": ytail_pool,
                          "mv": mvpool, "newt": npool, "outp": outp})
            # warm the ACT Exp table at t~0 so the 1.3us load is off the
            # first-exp critical path.
            warm = pools["newt"].tile([1, 1], F32, name="warm")
            nc.scalar.activation(warm[:], cs["em_t"][0:1, 0:1], EXP,
                                 bias=0.0, scale=1.0)
            # preamble issue order: dt0 K/Q fully first (they gate unit 0's
            # scores), then V kt0-7; V kt8-15 interleave into unit 0.
            vjobs = _v_jobs(nc, cf, io, cs, XT, W, V8, pools)
            kq0 = _kq_jobs(nc, cf, cs, W, XT, KTsc, QTsc, pools, 0)
            kq0[0]()
            kq0[2]()
            kq0[1]()
            kq0[3]()
            for _ in range(8):
                vjobs.pop(0)()
            _attn_epi(nc, cf, io, cs, KTsc, QTsc, V8, ctxT, pools, W, XT,
                      fold_gb, vjobs)

    nc.compile()
    return nc


# ---------------------------------------------------------------------------
_NC_CACHE = {}


def _get_nc(fold_gb):
    key = ("fold" if fold_gb else "full")
    if key not in _NC_CACHE:
        _NC_CACHE[key] = build_nc(fold_gb=fold_gb)
    return _NC_CACHE[key]


def make_in_maps(hidden_states, attention_mask, Wq, bq, Wk, bk, Wv, bv, Wo, bo,
                 ln_gamma, ln_beta):
    import ml_dtypes
    FP8NP = ml_dtypes.float8_e4m3

    hs = np.ascontiguousarray(np.asarray(hidden_states, dtype=np.float32))
    am = np.asarray(attention_mask, dtype=np.float32)
    B, S, H = hs.shape
    SQ = S // 2
    Wqf = np.asarray(Wq, np.float32); Wkf = np.asarray(Wk, np.float32)
    Wvf = np.asarray(Wv, np.float32); Wof = np.asarray(Wo, np.float32)
    bqf = np.asarray(bq, np.float32)
    bvf = np.asarray(bv, np.float32); bof = np.asarray(bo, np.float32)
    gf = np.asarray(ln_gamma, np.float32); btf = np.asarray(ln_beta, np.float32)
    fold_gb = bool(np.all(gf == 1.0) and np.all(btf == 0.0))

    def wkq_pack(Wm):
        # [p, dt, ks, j, m] = 16*W[(2ks+j)*128+p, dt*128+m]
        a = (16.0 * Wm).reshape(4, 2, P, 8, P).transpose(2, 3, 0, 1, 4)
        return np.ascontiguousarray(a.astype(FP8NP))

    def wvo_pack(Wm):
        # [p, ks, j, d] = 16*W[(2ks+j)*128+p, d]
        a = (16.0 * Wm).reshape(4, 2, P, H).transpose(2, 0, 1, 3)
        return np.ascontiguousarray(a.astype(FP8NP))

    common = {
        "wk": wkq_pack(Wkf), "wq": wkq_pack(Wqf),
        "wv": wvo_pack(Wvf), "wo": wvo_pack(Wof),
        "bq16": np.ascontiguousarray((16.0 * bqf).reshape(8, P).T),
    }
    if not fold_gb:
        common["gamma"] = gf
        common["beta"] = btf
    resid_const = (bof + bvf @ Wof).astype(np.float32)

    in_maps = []
    for c in range(8):
        b, qh = c // 2, c % 2
        xp = np.concatenate([hs[b, qh * SQ:(qh + 1) * SQ],
                             hs[b, (1 - qh) * SQ:(2 - qh) * SQ]], axis=0)
        mp = np.concatenate([am[b, 0, 0, qh * SQ:(qh + 1) * SQ],
                             am[b, 0, 0, (1 - qh) * SQ:(2 - qh) * SQ]], axis=0)
        xt8 = np.ascontiguousarray(
            xp.T.reshape(8, P, S).transpose(1, 0, 2).astype(FP8NP))
        em = np.exp(mp).astype(np.float32)             # [S]
        em_t = np.ascontiguousarray(em.reshape(16, P).T)
        em8 = np.ascontiguousarray(
            np.repeat(em_t.astype(FP8NP)[:, :, None], 16, axis=2))
        xres = np.ascontiguousarray((xp[0:SQ] + resid_const[None, :]).astype(ml_dtypes.bfloat16))
        in_maps.append({"xt8": xt8, "em_t": em_t, "em8": em8,
                        "xres": xres, **common})
    return in_maps, fold_gb


def kernel(hidden_states, attention_mask, Wq, bq, Wk, bk, Wv, bv, Wo, bo,
           ln_gamma, ln_beta):
    from concourse.bass_utils import run_bass_kernel_spmd

    B, S, H = np.asarray(hidden_states).shape
    SQ = S // 2
    in_maps, fold_gb = make_in_maps(hidden_states, attention_mask, Wq, bq,
                                    Wk, bk, Wv, bv, Wo, bo, ln_gamma, ln_beta)
    nc = _get_nc(fold_gb)
    res = run_bass_kernel_spmd(nc, in_maps, list(range(8)))
    kernel.last_results = res

    outp = np.empty((B, S, H), np.float32)
    for c in range(8):
        b, qh = c // 2, c % 2
        outp[b, qh * SQ:(qh + 1) * SQ] = res.results[c]["out"]
    return outp


# revision 80
# speedup vs baseline: 1.0084x; 1.0084x over previous
"""Trainium2 Bass kernel for CustomRobertaAttention (B=4, S=2048, H=1024, NH=16).

Sharding: 8 cores = (batch b, query-half qh), zero cross-core comms; host
permutes tokens per core so local queries are rows 0..1023 and gathers.

Numerics: fp8e4 (e4m3) DoubleRow matmuls everywhere (2x128 contraction packed
per instruction). Host pre-casts x and 16*W to fp8. Scale/bias folding:
  - bk dropped (softmax-invariant along keys), bq folded into Q drain
  - bv/bo folded into the residual stream (x'' = x + bo + bv@Wo, host side)
  - attention mask folded into V via em=exp(mask) (V drain scale + ones col)
  - exp arg scale 1/2048 absorbs the 16*16 weight prescale and 1/sqrt(64)
Softmax exp runs on ACT (true exp->fp8) and DVE (Schraudolph bit-trick
ts->int8 emitting the fp8 bit pattern); only these two engines can read PSUM,
so they are the bottleneck and all SBUF-side work (residual add, LN apply)
is pushed to Pool. When the inputs have ln_gamma==1 and ln_beta==0 (the
shipped setup_inputs), the gamma/beta ops are folded away entirely.
LayerNorm: bn_stats + batched Newton rsqrt.
"""
import numpy as np

import concourse.bass as bass
import concourse.bacc as bacc
import concourse.tile as tile
import concourse.mybir as mybir

F32 = mybir.dt.float32
BF16 = mybir.dt.bfloat16
FP8 = mybir.dt.float8e4
I8 = mybir.dt.int8
EXP = mybir.ActivationFunctionType.Exp
IDN = mybir.ActivationFunctionType.Identity
OP = mybir.AluOpType
DR = mybir.MatmulPerfMode.DoubleRow

P = 128
HD = 64
LN_EPS = 1e-12
LOG2E = 1.4426950408889634

# ---- engine assignment knobs -------------------------------------------------
# exp engine patterns: 8 kt-pair slots per unit; "A"=ACT true exp,
# "D"=DVE Schraudolph bit-trick.
EXP_PAT_QB0 = (("A", "D", "A", "D", "A", "A", "D", "A"),
               ("D", "A", "A", "D", "A", "A", "D", "A"))
EXP_PAT_QB1 = (("A", "D", "A", "A", "D", "A", "D", "A"),
               ("D", "A", "A", "D", "A", "A", "D", "A"))
EXP_PAT_LAST = ("A", "A", "D", "A", "A", "A", "D", "A")
V_ROT = ("A", "D")       # V projection pair-drain engines
K_ROT = ("A", "D")       # K projection pair-drain engines
Q_ROT = ("A", "D")       # Q projection pair-drain engines
EPI_DRAIN = ("A", "A")   # out-proj drain engine per 512-col half
APPLY_ENG = "D"          # LN apply engine (fold path)
RESID_ENG = "D"          # residual add engine
TAIL_RESID_ENG = "D"
TAIL_APPLY_ENG = "A"


class Cfg:
    def __init__(self, S, SQ, H, NH):
        self.S, self.SQ, self.H, self.NH = S, SQ, H, NH
        self.HT = H // P        # 8 h-tiles
        self.KS = self.HT // 2  # 4 DoubleRow contraction steps
        self.DT = H // P        # 8 d-tiles
        self.NKT = S // P       # 16 key tiles
        self.QB = 512
        self.NQB = SQ // self.QB
        self.NQT = SQ // P      # 8 query P-tiles


def _eng(nc, tag):
    return {"A": nc.scalar, "D": nc.vector, "P": nc.gpsimd}[tag]


def _drain(nc, tag, out, in0, mul=None, add=None):
    """PSUM->SBUF cast with optional per-partition [P,1] AP scale/bias.
    tag 'A' -> ACT Identity(in*mul + add); 'D' -> tensor_scalar."""
    if tag == "A":
        nc.scalar.activation(out, in0, IDN,
                             bias=add if add is not None else 0.0,
                             scale=mul if mul is not None else 1.0)
    else:
        eng = _eng(nc, tag)
        if mul is not None and add is not None:
            eng.tensor_scalar(out=out, in0=in0, scalar1=mul, scalar2=add,
                              op0=OP.mult, op1=OP.add)
        elif mul is not None:
            eng.tensor_scalar(out=out, in0=in0, scalar1=mul, scalar2=None,
                              op0=OP.mult)
        elif add is not None:
            eng.tensor_scalar(out=out, in0=in0, scalar1=add, scalar2=None,
                              op0=OP.add)
        else:
            eng.tensor_copy(out, in0)


def _consts(nc, cf, io, consts, fold_gb):
    cs = {}
    cs["em_t"] = consts.tile([P, cf.NKT], F32, name="em_t")
    nc.sync.dma_start(cs["em_t"][:], io["em_t"][:, :])
    cs["em8"] = consts.tile([P, cf.NKT, cf.NH], FP8, name="em8")
    nc.sync.dma_start(cs["em8"][:], io["em8"][:, :, :])
    cs["bq16"] = consts.tile([P, cf.DT], F32, name="bq16")
    nc.sync.dma_start(cs["bq16"][:], io["bq16"][:, :])
    if not fold_gb:
        for nm in ("gamma", "beta"):
            cs[nm] = consts.tile([P, cf.H], F32, name=nm)
            nc.sync.dma_start(cs[nm][:],
                              io[nm].ap().unsqueeze(0).partition_broadcast(P))
    return cs


def _v_jobs(nc, cf, io, cs, XT, W, V8, pools):
    """Build per-kt V projection closures: one [P,2,512] psum pair (shared
    psS ring) -> one 1024-col drain. Issued interleaved into units 0-1 so PE
    never stalls on a monolithic V phase. Also writes the em ones columns."""
    for kt in range(cf.NKT):
        nc.gpsimd.tensor_copy(V8[:, kt, :, HD], cs["em8"][:, kt, :])

    def mk(kt):
        def job():
            ps = pools["psS"].tile([P, 2, 512], F32, tag="psS")
            for dc in range(2):
                for ks in range(cf.KS):
                    nc.tensor.matmul(
                        ps[:, dc, :],
                        XT[:, 2 * ks:2 * ks + 2, kt * P:(kt + 1) * P],
                        W["wv"][:, ks, :, dc * 512:(dc + 1) * 512],
                        start=(ks == 0), stop=(ks == cf.KS - 1), perf_mode=DR)
            tag = V_ROT[kt % len(V_ROT)]
            _drain(nc, tag,
                   V8[:, kt, :, 0:HD],
                   ps[:].rearrange("p a b -> p (a b)")
                        .rearrange("p (h d) -> p h d", d=HD),
                   mul=cs["em_t"][:, kt:kt + 1])
        return job

    return [mk(kt) for kt in range(cf.NKT)]


def _kq_jobs(nc, cf, cs, W, XT, KTsc, QTsc, pools, dt):
    """K+Q projection jobs for one d-tile; paired 1024-col drains; merged
    scatter. Returns 4 closures so PE work interleaves with unit slots."""
    state = {}

    def k_pair(c2):
        def job():
            if "ktsb" not in state:
                state["ktsb"] = pools["ktsb"].tile([P, cf.S], FP8, tag="ktsb", name=f"ktsb{dt}")
            ktsb = state["ktsb"]
            ps = pools["psS"].tile([P, 2, 512], F32, tag="psS")
            for j2 in range(2):
                c = 2 * c2 + j2
                for ks in range(cf.KS):
                    nc.tensor.matmul(
                        ps[:, j2, :], W["wk"][:, dt, ks, :, :],
                        XT[:, 2 * ks:2 * ks + 2, c * 512:(c + 1) * 512],
                        start=(ks == 0), stop=(ks == cf.KS - 1), perf_mode=DR)
            _drain(nc, K_ROT[(dt + c2) % 2],
                   ktsb[:, c2 * 1024:(c2 + 1) * 1024],
                   ps[:].rearrange("p a b -> p (a b)"))
        return job

    def q_pair():
        def job():
            state["qtsb"] = pools["qtsb"].tile([P, cf.SQ], FP8, tag="qtsb", name=f"qtsb{dt}")
            qtsb = state["qtsb"]
            ps = pools["psS"].tile([P, 2, 512], F32, tag="psS")
            for j2 in range(2):
                for ks in range(cf.KS):
                    nc.tensor.matmul(
                        ps[:, j2, :], W["wq"][:, dt, ks, :, :],
                        XT[:, 2 * ks:2 * ks + 2, j2 * 512:(j2 + 1) * 512],
                        start=(ks == 0), stop=(ks == cf.KS - 1), perf_mode=DR)
            _drain(nc, Q_ROT[dt % 2],
                   qtsb[:, 0:1024],
                   ps[:].rearrange("p a b -> p (a b)"),
                   add=cs["bq16"][:, dt:dt + 1])
        return job

    def scatter():
        def job():
            for hp in range(2):
                h = 2 * dt + hp
                w, g = h % 4, h // 4
                nc.gpsimd.dma_start(KTsc[32 * w:32 * (w + 1), g, :, :],
                                    state["ktsb"][64 * hp:64 * hp + 64, :])
                nc.sync.dma_start(QTsc[32 * w:32 * (w + 1), g, :, :],
                                  state["qtsb"][64 * hp:64 * hp + 64, :])
        return job

    return [k_pair(0), k_pair(1), q_pair(), scatter()]


def _kq_dt(nc, cf, cs, W, XT, KTsc, QTsc, pools, dt):
    for job in _kq_jobs(nc, cf, cs, W, XT, KTsc, QTsc, pools, dt):
        job()


def _unit_step(nc, cf, pools, cs, KTsc, QTsc, V8, ctxT, h, qb, prev,
               vjobs=None, last=False, slot_jobs=None):
    """One attention unit, fine-grained: per pair-slot emit 2 score matmuls +
    exp + one ctx step of the PREVIOUS unit. Returns (h, qb, pairs, psc)."""
    w, g = h % 4, h // 4
    q0 = qb * cf.QB
    a_dve = 8.0 * LOG2E / 2048.0
    pats = EXP_PAT_QB0 if qb == 0 else EXP_PAT_QB1
    pat = pats[h % len(pats)]
    if last:
        pat = EXP_PAT_LAST
    slot_jobs = slot_jobs or {}
    pairs = []
    psc = pools["psC"].tile([HD + 1, 512], F32, tag="psC",
                            name=f"psc_{qb}_{h}")
    for i in range(8):
        tag = pat[i]
        pss = pools["psS"].tile([P, 2, 512], F32, tag="psS")
        for j in range(2):
            kt = 2 * i + j
            nc.tensor.matmul(
                pss[:, j, :],
                KTsc[32 * w:32 * (w + 1), g, :, kt * P:(kt + 1) * P],
                QTsc[32 * w:32 * (w + 1), g, :, q0:q0 + cf.QB],
                start=True, stop=True, perf_mode=DR,
                tile_position=(32 * w, 0))
        et = pools["exp"].tile([P, 2, 512], FP8, tag="exp")
        if tag == "A":
            nc.scalar.activation(
                et[:].rearrange("p a b -> p (a b)"),
                pss[:].rearrange("p a b -> p (a b)"),
                EXP, bias=0.0, scale=1.0 / 2048.0)
        else:
            nc.vector.tensor_scalar(
                out=et[:].rearrange("p a b -> p (a b)").bitcast(I8),
                in0=pss[:].rearrange("p a b -> p (a b)"),
                scalar1=a_dve, scalar2=56.0, op0=OP.mult, op1=OP.add)
        pairs.append(et[:])
        if vjobs:
            vjobs.pop(0)()
        for fn in slot_jobs.get(i, ()):
            fn()
        if prev is not None:
            ph, pqb, ppairs, ppsc = prev
            nc.tensor.matmul(
                ppsc[:], V8[:, 2 * i:2 * i + 2, ph, :], ppairs[i],
                start=(i == 0), stop=(i == 7), perf_mode=DR)
    return (h, qb, pairs, psc)


def _ctx_tail(nc, cf, pools, cs, V8, ctxT, prev):
    """Drain the last pending unit's ctx."""
    ph, pqb, ppairs, ppsc = prev
    for i in range(8):
        nc.tensor.matmul(
            ppsc[:], V8[:, 2 * i:2 * i + 2, ph, :], ppairs[i],
            start=(i == 0), stop=(i == 7), perf_mode=DR)


def _norm_start(nc, cf, pools, cs, unit):
    """Denominator -> SBUF -> recip -> DRAM -> bcast DMA.
    Returns the in-flight bcast tile; the TT-norm runs one unit later so the
    DMA chain never blocks an engine queue."""
    h, qb, _, psc = unit
    rec = pools["norm"].tile([1, 512], F32, tag="rec")
    nc.vector.reciprocal(rec[:], psc[HD:HD + 1, :])
    dst = pools["dram"].tile([1, 512], F32, tag="dst")
    nc.sync.dma_start(dst[:], rec[:])
    bc = pools["bcast"].tile([HD, 512], F32, tag="bc")
    nc.sync.dma_start(bc[:], dst[:].rearrange("a k -> (a k)")
                      .unsqueeze(0).partition_broadcast(HD))
    return bc


def _norm_finish(nc, cf, pools, ctxT, unit, bc):
    h, qb, _, psc = unit
    q0 = qb * cf.QB
    dt, hb = h // 2, 64 * (h % 2)
    if hb == 0:
        nc.vector.tensor_tensor(
            out=ctxT[0:HD, dt, q0:q0 + cf.QB], in0=psc[0:HD, :], in1=bc[:],
            op=OP.mult)
    else:
        ctmp = pools["norm"].tile([HD, 512], FP8, tag="ctmp")
        nc.vector.tensor_tensor(out=ctmp[:], in0=psc[0:HD, :], in1=bc[:],
                                op=OP.mult)
        nc.sync.dma_start(ctxT[HD:P, dt, q0:q0 + cf.QB], ctmp[:])


def _epi_oproj_pre(nc, cf, W, ctxT, pools, qt):
    """First 3 accumulation steps of the out-projection (dt 0-5 only) —
    issued early in the tail so PE stays warm while the last norms land."""
    ps = pools["psE"].tile([P, 2, 512], F32, tag="psS", name=f"pse{qt}")
    for c in range(2):
        for s in range(3):
            nc.tensor.matmul(
                ps[:, c, :], ctxT[:, 2 * s:2 * s + 2, qt * P:(qt + 1) * P],
                W["wo"][:, s, :, c * 512:(c + 1) * 512],
                start=(s == 0), stop=False, perf_mode=DR)
    return ps


def _epi_qt(nc, cf, io, cs, W, ctxT, pools, qt, grp, qi, tail=False,
            ps_pre=None):
    """Out-projection + residual for one query P-tile (stats issued
    separately via _epi_bn so the in-order DVE queue never stalls on the
    Pool residual add)."""
    psE = pools["psE"]
    yt, mv = grp
    if ps_pre is not None:
        ps = ps_pre
        for c in range(2):
            nc.tensor.matmul(
                ps[:, c, :], ctxT[:, 6:8, qt * P:(qt + 1) * P],
                W["wo"][:, 3, :, c * 512:(c + 1) * 512],
                start=False, stop=True, perf_mode=DR)
    else:
        ps = psE.tile([P, 2, 512], F32, tag="psS")
        for c in range(2):
            for s in range(4):
                nc.tensor.matmul(
                    ps[:, c, :], ctxT[:, 2 * s:2 * s + 2, qt * P:(qt + 1) * P],
                    W["wo"][:, s, :, c * 512:(c + 1) * 512],
                    start=(s == 0), stop=(s == 3), perf_mode=DR)
    if EPI_DRAIN[0] == EPI_DRAIN[1]:
        _drain(nc, EPI_DRAIN[0], yt[:, qi, :],
               ps[:].rearrange("p a b -> p (a b)"), mul=1.0 / 256.0)
    else:
        for c in range(2):
            _drain(nc, EPI_DRAIN[c], yt[:, qi, c * 512:(c + 1) * 512],
                   ps[:, c, :], mul=1.0 / 256.0)
    xr = pools["xres"].tile([P, cf.H], BF16, tag="xr")
    nc.sync.dma_start(xr[:], io["xres"][qt * P:(qt + 1) * P, :])
    if tail:
        rtag = ("D", "D", "D", "D")[qi % 4]
    else:
        rtag = RESID_ENG
    _eng(nc, rtag).tensor_tensor(out=yt[:, qi, :], in0=yt[:, qi, :],
                                 in1=xr[:], op=OP.add)
    if tail:
        _epi_bn(nc, cf, pools, grp, qi)


def _epi_bn(nc, cf, pools, grp, qi):
    yt, mv = grp
    stats = pools["mv"].tile([P, 2, nc.vector.BN_STATS_DIM], F32, tag="st")
    yv = yt[:, qi, :].rearrange("p (s f) -> p s f", s=2)
    for s in range(2):
        nc.vector.bn_stats(out=stats[:, s, :], in_=yv[:, s, :])
    nc.vector.bn_aggr(out=mv[:, qi, :], in_=stats[:])


def _epi_apply(nc, cf, io, cs, pools, grp, qts, fold_gb, tail=False):
    """Newton rsqrt (batched over len(qts) query tiles) + LN apply
    (+ gamma/beta when not folded) + out DMA."""
    yt, mv = grp
    n = len(qts)
    npool = pools["newt"]
    xv = npool.tile([P, n], F32, tag="xv", name=f"xv{qts[0]}")
    nc.vector.tensor_scalar(out=xv[:], in0=mv[:, 0:n, 1], scalar1=-0.5,
                            scalar2=None, op0=OP.mult)
    rstd = npool.tile([P, n], F32, tag="rstd", name=f"rstd{qts[0]}")
    nc.vector.memset(rstd[:], 1.0)
    tmp = npool.tile([P, n], F32, tag="tmp", name=f"tmp{qts[0]}")
    for _ in range(3):
        nc.vector.tensor_mul(tmp[:], rstd[:], rstd[:])
        nc.vector.tensor_mul(tmp[:], tmp[:], xv[:])
        nc.vector.tensor_scalar(out=tmp[:], in0=tmp[:], scalar1=1.5,
                                scalar2=None, op0=OP.add)
        nc.vector.tensor_mul(rstd[:], rstd[:], tmp[:])
    bias_t = npool.tile([P, n], F32, tag="bias_t", name=f"bias_t{qts[0]}")
    nc.vector.tensor_scalar(out=bias_t[:], in0=mv[:, 0:n, 0], scalar1=-1.0,
                            scalar2=None, op0=OP.mult)
    nc.vector.tensor_mul(bias_t[:], bias_t[:], rstd[:])
    for qi, qt in enumerate(qts):
        if tail:
            atag = ("D", "D")[qi % 2]
        else:
            atag = APPLY_ENG
        ob = pools["outp"].tile([P, cf.H], BF16, tag="ob", name=f"ob{qt}")
        if fold_gb:
            _drain(nc, atag, ob[:], yt[:, qi, :],
                   mul=rstd[:, qi:qi + 1], add=bias_t[:, qi:qi + 1])
        else:
            _drain(nc, atag, yt[:, qi, :], yt[:, qi, :],
                   mul=rstd[:, qi:qi + 1], add=bias_t[:, qi:qi + 1])
            nc.gpsimd.tensor_mul(yt[:, qi, :], yt[:, qi, :], cs["gamma"][:])
            nc.vector.tensor_tensor(out=ob[:], in0=yt[:, qi, :],
                                    in1=cs["beta"][:], op=OP.add)
        q = (nc.gpsimd if (tail and qi % 2 == 1) else nc.sync)
        q.dma_start(io["out"][qt * P:(qt + 1) * P, :], ob[:])


def _attn_epi(nc, cf, io, cs, KTsc, QTsc, V8, ctxT, pools, W, XT, fold_gb,
              vjobs):
    """Attention units (fine-grained pipeline) with qb0 epilogues interleaved
    into qb1, then the tail epilogues in pipelined pairs."""
    ypool, mvpool = pools["y"], pools["mv"]

    def new_grp(i):
        return (ypool.tile([P, 2, cf.H], BF16, name=f"y{i}", tag="y"),
                mvpool.tile([P, 2, 2], F32, name=f"mvg{i}", tag="mv"))

    grps = {}
    prev = None           # unit whose ctx interleaves into the current one
    norm_q = []           # units with ctx done, norm chain in flight
    pending = {}          # global unit idx -> {slot: [fns]} deferred work
    NU = cf.NQB * cf.NH

    def schedule(g, slot, fn):
        if g >= NU:
            fn()          # past the last unit: run at once (pre-tail)
        else:
            pending.setdefault(g, {}).setdefault(slot, []).append(fn)

    # qb1 runs odd heads first so the final units' norms skip the ctmp DMA
    # hop (even heads write ctxT directly).
    h_orders = (tuple(range(cf.NH)),
                tuple(range(1, cf.NH, 2)) + tuple(range(0, cf.NH, 2)))
    for qb in range(cf.NQB):
        for ui, h in enumerate(h_orders[qb]):
            g = qb * cf.NH + ui
            if qb == 0 and h % 2 == 0 and h // 2 + 1 < cf.DT:
                _kq_dt(nc, cf, cs, W, XT, KTsc, QTsc, pools, h // 2 + 1)
            cur = _unit_step(nc, cf, pools, cs, KTsc, QTsc, V8, ctxT,
                             h, qb, prev, vjobs=vjobs,
                             last=(qb == 1 and ui >= 14),
                             slot_jobs=pending.pop(g, None))
            if prev is not None:
                norm_q.append((prev, _norm_start(nc, cf, pools, cs, prev)))
            if len(norm_q) > 2:
                u, bc = norm_q.pop(0)
                schedule(g + 1, 3,
                         (lambda uu, bb: lambda: _norm_finish(
                             nc, cf, pools, ctxT, uu, bb))(u, bc))
            prev = cur
            if qb == 1 and ui % 4 == 3:
                qt = ui // 4
                pair = qt // 2
                if qt % 2 == 0:
                    grps[pair] = new_grp(pair)
                schedule(g + 1, 5,
                         (lambda q, gr: lambda: _epi_qt(
                             nc, cf, io, cs, W, ctxT, pools, q, gr,
                             q % 2))(qt, grps[pair]))
                schedule(g + 2, 2,
                         (lambda q, gr: lambda: _epi_bn(
                             nc, cf, pools, gr, q % 2))(qt, grps[pair]))
                if qt % 2 == 1:
                    schedule(g + 2, 6,
                             (lambda q, gr: lambda: _epi_apply(
                                 nc, cf, io, cs, pools, gr, (q - 1, q),
                                 fold_gb))(qt, grps[pair]))
    for g in sorted(pending):
        for slot in sorted(pending[g]):
            for fn in pending[g][slot]:
                fn()
    pending.clear()
    _ctx_tail(nc, cf, pools, cs, V8, ctxT, prev)
    pre = {}
    norm_q.append((prev, _norm_start(nc, cf, pools, cs, prev)))
    for u, bc in norm_q:
        _norm_finish(nc, cf, pools, ctxT, u, bc)
    tgrp = (pools["ytail"].tile([P, 4, cf.H], BF16, name="ytail"),
            pools["mv"].tile([P, 4, 2], F32, name="mvtail", tag="mvt"))
    tg0 = (tgrp[0][:, 0:2, :], tgrp[1][:, 0:2, :])
    tg1 = (tgrp[0][:, 2:4, :], tgrp[1][:, 2:4, :])
    for qt in (4, 5):
        _epi_qt(nc, cf, io, cs, W, ctxT, pools, qt, tg0, qt - 4, tail=True,
                ps_pre=pre.get(qt))
    _epi_apply(nc, cf, io, cs, pools, tg0, (4, 5), fold_gb, tail=True)
    for qt in (6, 7):
        _epi_qt(nc, cf, io, cs, W, ctxT, pools, qt, tg1, qt - 6, tail=True,
                ps_pre=pre.get(qt))
    _epi_apply(nc, cf, io, cs, pools, tg1, (6, 7), fold_gb, tail=True)


def build_nc(S=2048, SQ=1024, H=1024, NH=16, QB=512, num_devices=8,
             fold_gb=True):
    cf = Cfg(S, SQ, H, NH)
    nc = bacc.Bacc("TRN2", target_bir_lowering=False, debug=False,
                   num_devices=num_devices)

    io = {}
    io["xt8"] = nc.dram_tensor("xt8", [P, cf.HT, S], FP8, kind="ExternalInput")
    io["wk"] = nc.dram_tensor("wk", [P, cf.DT, cf.KS, 2, P], FP8,
                              kind="ExternalInput")
    io["wq"] = nc.dram_tensor("wq", [P, cf.DT, cf.KS, 2, P], FP8,
                              kind="ExternalInput")
    io["wv"] = nc.dram_tensor("wv", [P, cf.KS, 2, H], FP8,
                              kind="ExternalInput")
    io["wo"] = nc.dram_tensor("wo", [P, cf.KS, 2, H], FP8,
                              kind="ExternalInput")
    io["em_t"] = nc.dram_tensor("em_t", [P, cf.NKT], F32, kind="ExternalInput")
    io["em8"] = nc.dram_tensor("em8", [P, cf.NKT, NH], FP8,
                               kind="ExternalInput")
    io["bq16"] = nc.dram_tensor("bq16", [P, cf.DT], F32, kind="ExternalInput")
    io["xres"] = nc.dram_tensor("xres", [SQ, H], BF16, kind="ExternalInput")
    if not fold_gb:
        io["gamma"] = nc.dram_tensor("gamma", [H], F32, kind="ExternalInput")
        io["beta"] = nc.dram_tensor("beta", [H], F32, kind="ExternalInput")
    io["out"] = nc.dram_tensor("out", [SQ, H], BF16, kind="ExternalOutput")

    with tile.TileContext(nc) as tc, \
         tc.tile_pool(name="consts", bufs=1) as consts, \
         tc.tile_pool(name="xt", bufs=1) as xt_pool, \
         tc.tile_pool(name="wsb", bufs=1) as wsb, \
         tc.tile_pool(name="ktsc", bufs=1) as ktsc_pool, \
         tc.tile_pool(name="qtsc", bufs=1) as qtsc_pool, \
         tc.tile_pool(name="vv", bufs=1) as v_pool, \
         tc.tile_pool(name="ctx", bufs=1) as ctx_pool, \
         tc.tile_pool(name="dram", bufs=4, space="DRAM") as dram_pool:
        XT = xt_pool.tile([P, cf.HT, S], FP8)
        W = {}
        for nm in ("wv", "wk", "wq", "wo"):
            shp = [P, cf.DT, cf.KS, 2, P] if nm in ("wk", "wq") \
                else [P, cf.KS, 2, H]
            W[nm] = wsb.tile(shp, FP8, name=nm)
        # load order tuned for fast start: xt chunk0 + dt0 K/Q weights first
        # so the dt0 K/Q projections can begin ASAP, then V, then the rest.
        nc.sync.dma_start(XT[:, :, 0:512], io["xt8"][:, :, 0:512])
        nc.sync.dma_start(W["wk"][:, 0:1, :, :, :], io["wk"][:, 0:1, :, :, :])
        nc.sync.dma_start(W["wq"][:, 0:1, :, :, :], io["wq"][:, 0:1, :, :, :])
        for c in range(1, 4):
            nc.sync.dma_start(XT[:, :, c * 512:(c + 1) * 512],
                              io["xt8"][:, :, c * 512:(c + 1) * 512])
        nc.sync.dma_start(W["wv"][:], io["wv"][...])
        cs = _consts(nc, cf, io, consts, fold_gb)
        nc.sync.dma_start(W["wk"][:, 1:4, :, :, :], io["wk"][:, 1:4, :, :, :])
        nc.sync.dma_start(W["wq"][:, 1:4, :, :, :], io["wq"][:, 1:4, :, :, :])
        nc.sync.dma_start(W["wk"][:, 4:8, :, :, :], io["wk"][:, 4:8, :, :, :])
        nc.sync.dma_start(W["wq"][:, 4:8, :, :, :], io["wq"][:, 4:8, :, :, :])
        nc.sync.dma_start(W["wo"][:], io["wo"][...])
        KTsc = ktsc_pool.tile([P, 4, 2, S], FP8)
        QTsc = qtsc_pool.tile([P, 4, 2, SQ], FP8)
        V8 = v_pool.tile([P, cf.NKT, NH, HD + 1], FP8)
        ctxT = ctx_pool.tile([P, cf.DT, SQ], FP8)
        pools = {"dram": dram_pool}

        from contextlib import ExitStack
        with ExitStack() as stack:
            psS = stack.enter_context(tc.tile_pool(name="psS", bufs=3, space="PSUM"))
            psC = stack.enter_context(tc.tile_pool(name="psC", bufs=2, space="PSUM"))
            exp_pool = stack.enter_context(tc.tile_pool(name="exp", bufs=24))
            ktsb_pool = stack.enter_context(tc.tile_pool(name="ktsb", bufs=2))
            qtsb_pool = stack.enter_context(tc.tile_pool(name="qtsb", bufs=2))
            norm_pool = stack.enter_context(tc.tile_pool(name="norm", bufs=4))
            bcast_pool = stack.enter_context(tc.tile_pool(name="bcast", bufs=4))
            ypool = stack.enter_context(tc.tile_pool(name="y", bufs=2))
            ytail_pool = stack.enter_context(tc.tile_pool(name="ytail", bufs=1))
            xpool = stack.enter_context(tc.tile_pool(name="xres", bufs=3))
            mvpool = stack.enter_context(tc.tile_pool(name="mv", bufs=3))
            npool = stack.enter_context(tc.tile_pool(name="newt", bufs=2))
            outp = stack.enter_context(tc.tile_pool(name="outp", bufs=4))
            pools.update({"ktsb": ktsb_pool, "qtsb": qtsb_pool})
            pools.update({"psS": psS, "psC": psC, "psE": psS,
                          "exp": exp_pool, "norm": norm_pool,
                          "bcast": bcast_pool, "y": ypool, "xres": xpool,
                          "ytail": ytail_pool,
                          "mv": mvpool, "newt": npool, "outp": outp})
            # warm the ACT Exp table at t~0 so the 1.3us load is off the
            # first-exp critical path.
            warm = pools["newt"].tile([1, 1], F32, name="warm")
            nc.scalar.activation(warm[:], cs["em_t"][0:1, 0:1], EXP,
                                 bias=0.0, scale=1.0)
            # preamble issue order: dt0 K/Q fully first (they gate unit 0's
            # scores), then V kt0-7; V kt8-15 interleave into unit 0.
            vjobs = _v_jobs(nc, cf, io, cs, XT, W, V8, pools)
            kq0 = _kq_jobs(nc, cf, cs, W, XT, KTsc, QTsc, pools, 0)
            kq0[0]()
            kq0[2]()
            kq0[1]()
            kq0[3]()
            for _ in range(8):
                vjobs.pop(0)()
            _attn_epi(nc, cf, io, cs, KTsc, QTsc, V8, ctxT, pools, W, XT,
                      fold_gb, vjobs)

    nc.compile()
    return nc


# ---------------------------------------------------------------------------
_NC_CACHE = {}


def _get_nc(fold_gb):
    key = ("fold" if fold_gb else "full")
    if key not in _NC_CACHE:
        _NC_CACHE[key] = build_nc(fold_gb=fold_gb)
    return _NC_CACHE[key]


def make_in_maps(hidden_states, attention_mask, Wq, bq, Wk, bk, Wv, bv, Wo, bo,
                 ln_gamma, ln_beta):
    import ml_dtypes
    FP8NP = ml_dtypes.float8_e4m3

    hs = np.ascontiguousarray(np.asarray(hidden_states, dtype=np.float32))
    am = np.asarray(attention_mask, dtype=np.float32)
    B, S, H = hs.shape
    SQ = S // 2
    Wqf = np.asarray(Wq, np.float32); Wkf = np.asarray(Wk, np.float32)
    Wvf = np.asarray(Wv, np.float32); Wof = np.asarray(Wo, np.float32)
    bqf = np.asarray(bq, np.float32)
    bvf = np.asarray(bv, np.float32); bof = np.asarray(bo, np.float32)
    gf = np.asarray(ln_gamma, np.float32); btf = np.asarray(ln_beta, np.float32)
    fold_gb = bool(np.all(gf == 1.0) and np.all(btf == 0.0))

    def wkq_pack(Wm):
        # [p, dt, ks, j, m] = 16*W[(2ks+j)*128+p, dt*128+m]
        a = (16.0 * Wm).reshape(4, 2, P, 8, P).transpose(2, 3, 0, 1, 4)
        return np.ascontiguousarray(a.astype(FP8NP))

    def wvo_pack(Wm):
        # [p, ks, j, d] = 16*W[(2ks+j)*128+p, d]
        a = (16.0 * Wm).reshape(4, 2, P, H).transpose(2, 0, 1, 3)
        return np.ascontiguousarray(a.astype(FP8NP))

    common = {
        "wk": wkq_pack(Wkf), "wq": wkq_pack(Wqf),
        "wv": wvo_pack(Wvf), "wo": wvo_pack(Wof),
        "bq16": np.ascontiguousarray((16.0 * bqf).reshape(8, P).T),
    }
    if not fold_gb:
        common["gamma"] = gf
        common["beta"] = btf
    resid_const = (bof + bvf @ Wof).astype(np.float32)

    in_maps = []
    for c in range(8):
        b, qh = c // 2, c % 2
        xp = np.concatenate([hs[b, qh * SQ:(qh + 1) * SQ],
                             hs[b, (1 - qh) * SQ:(2 - qh) * SQ]], axis=0)
        mp = np.concatenate([am[b, 0, 0, qh * SQ:(qh + 1) * SQ],
                             am[b, 0, 0, (1 - qh) * SQ:(2 - qh) * SQ]], axis=0)
        xt8 = np.ascontiguousarray(
            xp.T.reshape(8, P, S).transpose(1, 0, 2).astype(FP8NP))
        em = np.exp(mp).astype(np.float32)             # [S]
        em_t = np.ascontiguousarray(em.reshape(16, P).T)
        em8 = np.ascontiguousarray(
            np.repeat(em_t.astype(FP8NP)[:, :, None], 16, axis=2))
        xres = np.ascontiguousarray((xp[0:SQ] + resid_const[None, :]).astype(ml_dtypes.bfloat16))
        in_maps.append({"xt8": xt8, "em_t": em_t, "em8": em8,
                        "xres": xres, **common})
    return in_maps, fold_gb


def kernel(hidden_states, attention_mask, Wq, bq, Wk, bk, Wv, bv, Wo, bo,
           ln_gamma, ln_beta):
    from concourse.bass_utils import run_bass_kernel_spmd

    B, S, H = np.asarray(hidden_states).shape
    SQ = S // 2
    in_maps, fold_gb = make_in_maps(hidden_states, attention_mask, Wq, bq,
                                    Wk, bk, Wv, bv, Wo, bo, ln_gamma, ln_beta)
    nc = _get_nc(fold_gb)
    res = run_bass_kernel_spmd(nc, in_maps, list(range(8)))
    kernel.last_results = res

    outp = np.empty((B, S, H), np.float32)
    for c in range(8):
        b, qh = c // 2, c % 2
        outp[b, qh * SQ:(qh + 1) * SQ] = res.results[c]["out"]
    return outp
